# revision 8
# baseline (speedup 1.0000x reference)
"""DecoderPooler kernel for Trainium2 (Bass), 8-core data-parallel.

Problem: given hidden_state [16, 4096, 1024] f32 and attention_mask
[16, 4096] int32 (contiguous prefix of ones), return the hidden vector at
the last valid position of each sequence: out[b] = hidden[b, sum(mask[b])-1].

Strategy: shard the batch dim (16) across 8 cores, 2 sequences/core.
Each core reads only its mask rows (~17 KB as bf16) and the two needed
H-vectors (8 KB) from HBM — never the other 32 MB of its hidden_state
shard.  The critical path is a minimal serial chain:

  1. Host lays the core's two mask rows out as a [128, 68] bf16 tile:
     sequence b's 4096 mask values sit at columns [34b, 34b+32) (32 values
     per partition, partition-major over all 128 partitions), and two bias
     columns per sequence on partition 0 encode b*S - 1 as bf16-exact
     constants: seq 0 has {0, -1}, seq 1 has {+4096, -1}.
  2. SP issues one DMA for the tile; PE waits on its completion
     semaphore and consumes the tile directly with a ones-vector matmul:
     psum[1, 2, 34] = ones[128,1].T @ work[128,68] (per-column sums).
     (A DGE drain does NOT guarantee the transfer landed — verified on
     device — so the completion semaphore is the only safe gate.)
  3. DVE does ONE fused 3-D reduce+cast on partition 0:
     idx[0, b] = int32(sum(psum[0, b, :])) = b*S + len_b - 1.
  4. SP (seq 0) and ACT (seq 1) each reg_load their shard row index from
     SBUF and issue a register-dynamically-addressed DRAM->DRAM DMA
     copying hidden[idx[b], :] straight into the output row, then wait on
     the stores' completion semaphore.

Raw Bass (no TileContext): the kernel is a short serial chain, and
explicit semaphores keep the tail free of the multi-sem drain that
overflows this walrus build's per-instruction sync-wait limit.
"""

import numpy as np

import concourse.bass as bass
import concourse.mybir as mybir
from concourse.bass_utils import run_bass_kernel_spmd

B, S, H = 16, 4096, 1024
N_CORES = 8
B_PER = B // N_CORES  # 2 sequences per core
PARTS = 128  # partitions used by the mask tile
CHUNK = S // PARTS  # 32 mask elements per partition per sequence
SEQC = CHUNK + 2  # columns per sequence incl. two bias columns
WCOLS = B_PER * SEQC  # 68

_NC_CACHE = None


def build_bass(reps: int = 1) -> bass.Bass:
    """Per-core program: gather the last valid token of B_PER sequences.

    reps>1 repeats the chain serially (same tiles, cumulative semaphore
    thresholds) — used only for on-device timing by delta: the per-rep
    increment of wall time is the HW kernel latency, with host/RPC/launch
    overhead cancelled out.
    """
    nc = bass.Bass()
    # flat [B_PER*S, H] view of this core's hidden_state shard
    hidden = nc.declare_dram_parameter(
        "hidden", [B_PER * S, H], mybir.dt.float32, isOutput=False
    )
    # host-prepped [128, 68] bf16 mask layout (see module docstring)
    mask = nc.declare_dram_parameter(
        "mask", [PARTS, WCOLS], mybir.dt.bfloat16, isOutput=False
    )
    out = nc.declare_dram_parameter("out", [B_PER, H], mybir.dt.float32, isOutput=True)

    with (
        nc.sbuf_tensor([PARTS, WCOLS], mybir.dt.bfloat16) as work,
        nc.sbuf_tensor([PARTS, 1], mybir.dt.bfloat16) as ones,
        nc.sbuf_tensor([1, B_PER], mybir.dt.int32) as idx,
        nc.psum_tensor([1, B_PER, SEQC], mybir.dt.float32) as psum,
        nc.semaphore() as dma_sem,  # mask DMA completion -> PE
        nc.semaphore() as s_sem,  # store DMA completions -> SP/ACT tails
        nc.semaphore() as pe_sem,  # PE matmul -> DVE
        nc.semaphore() as v_sem,  # DVE reduce -> SP+ACT: idx ready
        nc.semaphore() as sel_sem,  # DVE ones memset -> PE (once)
        nc.Block() as block,
    ):

        @block.sync
        def _(sync):
            r0 = sync.alloc_register("r0")
            for i in range(reps):
                sync.dma_start(out=work[:], in_=mask[:]).then_inc(dma_sem, 16)
                # idx ready in SBUF
                sync.wait_ge(v_sem, i + 1)
                sync.reg_load(r0, idx[0:1, 0:1])
                # donate: the snap aliases the register instead of allocating
                # a fresh snapshot register per rep (the DMA descriptor
                # captures the value at issue, so reuse next rep is safe
                # behind the s_sem wait)
                v0 = sync.snap(r0, donate=True)
                sync.dma_start(
                    out=out[0:1, :], in_=hidden[bass.ds(v0, 1), :]
                ).then_inc(s_sem, 16)
                # don't let the program retire (or the next rep start)
                # with either store still in flight (scalar's included)
                sync.wait_ge(s_sem, 32 * (i + 1))

        @block.scalar
        def _(scalar):
            # second output row handled by ACT's sequencer in parallel with SP
            r1 = scalar.alloc_register("r1")
            for i in range(reps):
                scalar.wait_ge(v_sem, i + 1)
                # idx[0,1] already includes the +S shard-row offset, baked
                # into the bias columns on the host side
                scalar.reg_load(r1, idx[0:1, 1:2])
                v1 = scalar.snap(r1, donate=True)
                scalar.dma_start(
                    out=out[1:2, :], in_=hidden[bass.ds(v1, 1), :]
                ).then_inc(s_sem, 16)
                scalar.wait_ge(s_sem, 32 * (i + 1))

        @block.vector
        def _(vector):
            # ones column for the contraction; written once before the first
            # matmul (PE waits on sel_sem)
            vector.memset(ones[:], 1.0).then_inc(sel_sem, 1)
            for i in range(reps):
                vector.wait_ge(pe_sem, i + 1)
                # idx[0, b] = int32(sum(psum[0, b, :])) = len_b - 1; exact
                # (small integers in f32, converted on the DVE output stage).
                # One 3-D X-axis reduce handles both sequences and carries the
                # single sem update this walrus build allows per instruction.
                with nc.allow_low_precision(
                    reason="sum of 0/1 mask values; exact in int32"
                ):
                    vector.reduce_sum(
                        out=idx[0:1, 0:B_PER],
                        in_=psum[0:1, :, :],
                        axis=mybir.AxisListType.X,
                    ).then_inc(v_sem, 1)

        @block.tensor
        def _(tensor):
            tensor.wait_ge(sel_sem, 1)
            for i in range(reps):
                tensor.wait_ge(dma_sem, 16 * (i + 1))
                # psum[0, b, c] = sum_p work[p, 34b+c]: 68 column sums
                nc.tensor.matmul(
                    out=psum[0:1, :, :],
                    lhsT=ones[:],
                    rhs=work[:],
                    start=True,
                    stop=True,
                ).then_inc(pe_sem, 1)

    return nc


def build_bass_loop(n_iters: int) -> bass.Bass:
    """Timing build: the unrolled cumulative-threshold chain.  Semaphore
    counters were verified on-device not to wrap at 16 bits (reps=4000,
    s_sem up to 128000, bit-exact result), so plain build_bass(reps=N)
    serves for arbitrary N."""
    return build_bass(reps=n_iters)


def _get_nc() -> bass.Bass:
    global _NC_CACHE
    if _NC_CACHE is None:
        _NC_CACHE = build_bass()
    return _NC_CACHE


def _prep_mask(mask_rows: np.ndarray) -> np.ndarray:
    """[B_PER, S] 0/1 mask -> [128, 68] bf16 tile (see module docstring)."""
    bf16 = mybir.dt.np(mybir.dt.bfloat16)
    tile = np.zeros((PARTS, WCOLS), dtype=bf16)
    for b in range(B_PER):
        m = np.asarray(mask_rows[b]).reshape(PARTS, CHUNK).astype(bf16)
        tile[:, b * SEQC : b * SEQC + CHUNK] = m
        # bias columns: b*S - 1 split into bf16-exact terms
        tile[0, b * SEQC + CHUNK] = float(b * S)  # 0.0 or 4096.0
        tile[0, b * SEQC + CHUNK + 1] = -1.0
    return np.ascontiguousarray(tile)


def _shard_inputs(hidden_state: np.ndarray, attention_mask: np.ndarray):
    in_maps = []
    for c in range(N_CORES):
        lo, hi = c * B_PER, (c + 1) * B_PER
        hs = np.ascontiguousarray(
            hidden_state[lo:hi].reshape(B_PER * S, H), dtype=np.float32
        )
        in_maps.append({"hidden": hs, "mask": _prep_mask(attention_mask[lo:hi])})
    return in_maps


def run(hidden_state: np.ndarray, attention_mask: np.ndarray, **spmd_kwargs):
    """Run on 8 NeuronCores; returns (full_output, BassKernelResults)."""
    nc = _get_nc()
    in_maps = _shard_inputs(np.asarray(hidden_state), np.asarray(attention_mask))
    res = run_bass_kernel_spmd(nc, in_maps, core_ids=list(range(N_CORES)), **spmd_kwargs)
    out = np.concatenate([r["out"] for r in res.results], axis=0)
    return out, res


def kernel(hidden_state: np.ndarray, attention_mask: np.ndarray) -> np.ndarray:
    out, _ = run(hidden_state, attention_mask)
    return out


# revision 9
# speedup vs baseline: 1.0225x; 1.0225x over previous
"""DecoderPooler kernel for Trainium2 (Bass), 8-core data-parallel.

Problem: given hidden_state [16, 4096, 1024] f32 and attention_mask
[16, 4096] int32 (contiguous prefix of ones), return the hidden vector at
the last valid position of each sequence: out[b] = hidden[b, sum(mask[b])-1].

Strategy: shard the batch dim (16) across 8 cores, 2 sequences/core.
Each core reads only its mask rows (~17 KB as bf16) and the two needed
H-vectors (8 KB) from HBM — never the other 32 MB of its hidden_state
shard.  The critical path is a minimal serial chain:

  1. Host lays the core's two mask rows out as a [128, 68] bf16 tile:
     sequence b's 4096 mask values sit at columns [34b, 34b+32) (32 values
     per partition, partition-major over all 128 partitions), and two bias
     columns per sequence on partition 0 encode b*S - 1 as bf16-exact
     constants: seq 0 has {0, -1}, seq 1 has {+4096, -1}.
  2. SP issues one DMA for the tile; PE waits on its completion
     semaphore and consumes the tile directly with a ones-vector matmul:
     psum[1, 2, 34] = ones[128,1].T @ work[128,68] (per-column sums).
     (A DGE drain does NOT guarantee the transfer landed — verified on
     device — so the completion semaphore is the only safe gate.)
  3. DVE does ONE fused 3-D reduce+cast on partition 0:
     idx[0, b] = int32(sum(psum[0, b, :])) = b*S + len_b - 1.
  4. SP (seq 0) and ACT (seq 1) each reg_load their shard row index from
     SBUF and issue a register-dynamically-addressed DRAM->DRAM DMA
     copying hidden[idx[b], :] straight into the output row, then wait on
     the stores' completion semaphore.

Raw Bass (no TileContext): the kernel is a short serial chain, and
explicit semaphores keep the tail free of the multi-sem drain that
overflows this walrus build's per-instruction sync-wait limit.
"""

import numpy as np

import concourse.bass as bass
import concourse.mybir as mybir
from concourse.bass_utils import run_bass_kernel_spmd

B, S, H = 16, 4096, 1024
N_CORES = 8
B_PER = B // N_CORES  # 2 sequences per core
PARTS = 128  # partitions used by the mask tile
CHUNK = S // PARTS  # 32 mask elements per partition per sequence
SEQC = CHUNK + 2  # columns per sequence incl. two bias columns
WCOLS = B_PER * SEQC  # 68

_NC_CACHE = None


def build_bass(reps: int = 1) -> bass.Bass:
    """Per-core program: gather the last valid token of B_PER sequences.

    reps>1 repeats the chain serially (same tiles, cumulative semaphore
    thresholds) — used only for on-device timing by delta: the per-rep
    increment of wall time is the HW kernel latency, with host/RPC/launch
    overhead cancelled out.
    """
    nc = bass.Bass()
    # flat [B_PER*S, H] view of this core's hidden_state shard
    hidden = nc.declare_dram_parameter(
        "hidden", [B_PER * S, H], mybir.dt.float32, isOutput=False
    )
    # host-prepped [128, 68] bf16 mask layout (see module docstring)
    mask = nc.declare_dram_parameter(
        "mask", [PARTS, WCOLS], mybir.dt.bfloat16, isOutput=False
    )
    out = nc.declare_dram_parameter("out", [B_PER, H], mybir.dt.float32, isOutput=True)

    with (
        nc.sbuf_tensor([PARTS, WCOLS], mybir.dt.bfloat16) as work,
        nc.sbuf_tensor([PARTS, 1], mybir.dt.bfloat16) as ones,
        nc.sbuf_tensor([1, B_PER], mybir.dt.int32) as idx,
        nc.psum_tensor([1, B_PER, SEQC], mybir.dt.float32) as psum,
        nc.semaphore() as dma_sem,  # mask DMA completion -> PE
        nc.semaphore() as s_sem,  # store DMA completions -> SP/ACT tails
        nc.semaphore() as pe_sem,  # PE matmul -> DVE
        nc.semaphore() as v_sem,  # DVE reduce -> SP+ACT: idx ready
        nc.semaphore() as sel_sem,  # DVE ones memset -> PE (once)
        nc.Block() as block,
    ):

        @block.sync
        def _(sync):
            r0 = sync.alloc_register("r0")
            for i in range(reps):
                sync.dma_start(out=work[:], in_=mask[:]).then_inc(dma_sem, 16)
                # idx ready in SBUF
                sync.wait_ge(v_sem, i + 1)
                sync.reg_load(r0, idx[0:1, 0:1])
                # donate: the snap aliases the register instead of allocating
                # a fresh snapshot register per rep (the DMA descriptor
                # captures the value at issue, so reuse next rep is safe
                # behind the s_sem wait)
                v0 = sync.snap(r0, donate=True)
                sync.dma_start(
                    out=out[0:1, :], in_=hidden[bass.ds(v0, 1), :]
                ).then_inc(s_sem, 16)
                # don't let the program retire (or the next rep start)
                # with either store still in flight (scalar's included)
                sync.wait_ge(s_sem, 32 * (i + 1))

        @block.scalar
        def _(scalar):
            # second output row handled by ACT's sequencer in parallel with SP
            r1 = scalar.alloc_register("r1")
            for i in range(reps):
                scalar.wait_ge(v_sem, i + 1)
                # idx[0,1] already includes the +S shard-row offset, baked
                # into the bias columns on the host side
                scalar.reg_load(r1, idx[0:1, 1:2])
                v1 = scalar.snap(r1, donate=True)
                scalar.dma_start(
                    out=out[1:2, :], in_=hidden[bass.ds(v1, 1), :]
                ).then_inc(s_sem, 16)
                # no tail wait on ACT: SP's s_sem >= 32(i+1) wait covers both
                # stores (program-end safety and rep serialization), and ACT's
                # next rep is gated through v_sem behind that wait anyway

        @block.vector
        def _(vector):
            # ones column for the contraction; written once before the first
            # matmul (PE waits on sel_sem)
            vector.memset(ones[:], 1.0).then_inc(sel_sem, 1)
            for i in range(reps):
                vector.wait_ge(pe_sem, i + 1)
                # idx[0, b] = int32(sum(psum[0, b, :])) = len_b - 1; exact
                # (small integers in f32, converted on the DVE output stage).
                # One 3-D X-axis reduce handles both sequences and carries the
                # single sem update this walrus build allows per instruction.
                with nc.allow_low_precision(
                    reason="sum of 0/1 mask values; exact in int32"
                ):
                    vector.reduce_sum(
                        out=idx[0:1, 0:B_PER],
                        in_=psum[0:1, :, :],
                        axis=mybir.AxisListType.X,
                    ).then_inc(v_sem, 1)

        @block.tensor
        def _(tensor):
            tensor.wait_ge(sel_sem, 1)
            for i in range(reps):
                tensor.wait_ge(dma_sem, 16 * (i + 1))
                # psum[0, b, c] = sum_p work[p, 34b+c]: 68 column sums
                nc.tensor.matmul(
                    out=psum[0:1, :, :],
                    lhsT=ones[:],
                    rhs=work[:],
                    start=True,
                    stop=True,
                ).then_inc(pe_sem, 1)

    return nc


def build_bass_loop(n_iters: int) -> bass.Bass:
    """Timing build: the unrolled cumulative-threshold chain.  Semaphore
    counters were verified on-device not to wrap at 16 bits (reps=4000,
    s_sem up to 128000, bit-exact result), so plain build_bass(reps=N)
    serves for arbitrary N."""
    return build_bass(reps=n_iters)


def _get_nc() -> bass.Bass:
    global _NC_CACHE
    if _NC_CACHE is None:
        _NC_CACHE = build_bass()
    return _NC_CACHE


def _prep_mask(mask_rows: np.ndarray) -> np.ndarray:
    """[B_PER, S] 0/1 mask -> [128, 68] bf16 tile (see module docstring)."""
    bf16 = mybir.dt.np(mybir.dt.bfloat16)
    tile = np.zeros((PARTS, WCOLS), dtype=bf16)
    for b in range(B_PER):
        m = np.asarray(mask_rows[b]).reshape(PARTS, CHUNK).astype(bf16)
        tile[:, b * SEQC : b * SEQC + CHUNK] = m
        # bias columns: b*S - 1 split into bf16-exact terms
        tile[0, b * SEQC + CHUNK] = float(b * S)  # 0.0 or 4096.0
        tile[0, b * SEQC + CHUNK + 1] = -1.0
    return np.ascontiguousarray(tile)


def _shard_inputs(hidden_state: np.ndarray, attention_mask: np.ndarray):
    in_maps = []
    for c in range(N_CORES):
        lo, hi = c * B_PER, (c + 1) * B_PER
        hs = np.ascontiguousarray(
            hidden_state[lo:hi].reshape(B_PER * S, H), dtype=np.float32
        )
        in_maps.append({"hidden": hs, "mask": _prep_mask(attention_mask[lo:hi])})
    return in_maps


def run(hidden_state: np.ndarray, attention_mask: np.ndarray, **spmd_kwargs):
    """Run on 8 NeuronCores; returns (full_output, BassKernelResults)."""
    nc = _get_nc()
    in_maps = _shard_inputs(np.asarray(hidden_state), np.asarray(attention_mask))
    res = run_bass_kernel_spmd(nc, in_maps, core_ids=list(range(N_CORES)), **spmd_kwargs)
    out = np.concatenate([r["out"] for r in res.results], axis=0)
    return out, res


def kernel(hidden_state: np.ndarray, attention_mask: np.ndarray) -> np.ndarray:
    out, _ = run(hidden_state, attention_mask)
    return out


# revision 10
# speedup vs baseline: 1.1550x; 1.1295x over previous
"""DecoderPooler kernel for Trainium2 (Bass), 8-core data-parallel.

Problem: given hidden_state [16, 4096, 1024] f32 and attention_mask
[16, 4096] int32 (contiguous prefix of ones), return the hidden vector at
the last valid position of each sequence: out[b] = hidden[b, sum(mask[b])-1].

Strategy: shard the batch dim (16) across 8 cores, 2 sequences/core.
Each core reads only its mask rows (~17 KB as bf16) and the two needed
H-vectors (8 KB) from HBM — never the other 32 MB of its hidden_state
shard.  The critical path is a minimal serial chain:

  1. Host lays the core's two mask rows out as a [128, 68] bf16 tile:
     sequence b's 4096 mask values sit at columns [34b, 34b+32) (32 values
     per partition, partition-major over all 128 partitions), and two bias
     columns per sequence on partition 0 encode b*S - 1 as bf16-exact
     constants: seq 0 has {0, -1}, seq 1 has {+4096, -1}.
  2. SP issues one DMA for the tile; PE waits on its completion
     semaphore and consumes the tile directly with a ones-vector matmul:
     psum[1, 2, 34] = ones[128,1].T @ work[128,68] (per-column sums).
     (A DGE drain does NOT guarantee the transfer landed — verified on
     device — so the completion semaphore is the only safe gate.)
  3. DVE does ONE fused 3-D reduce+cast on partition 0:
     idx[0, b] = int32(sum(psum[0, b, :])) = b*S + len_b - 1.
  4. SP (seq 0) and ACT (seq 1) each reg_load their shard row index from
     SBUF and issue a register-dynamically-addressed DRAM->DRAM DMA
     copying hidden[idx[b], :] straight into the output row, then wait on
     the stores' completion semaphore.

Raw Bass (no TileContext): the kernel is a short serial chain, and
explicit semaphores keep the tail free of the multi-sem drain that
overflows this walrus build's per-instruction sync-wait limit.
"""

import numpy as np

import concourse.bass as bass
import concourse.mybir as mybir
from concourse.bass_utils import run_bass_kernel_spmd

B, S, H = 16, 4096, 1024
N_CORES = 8
B_PER = B // N_CORES  # 2 sequences per core
PARTS = 128  # partitions used by the mask tile
CHUNK = S // PARTS  # 32 mask elements per partition per sequence
SEQC = CHUNK + 2  # columns per sequence incl. two bias columns
WCOLS = B_PER * SEQC  # 68

_NC_CACHE = None


def build_bass(reps: int = 1) -> bass.Bass:
    """Per-core program: gather the last valid token of B_PER sequences.

    reps>1 repeats the chain serially (same tiles, cumulative semaphore
    thresholds) — used only for on-device timing by delta: the per-rep
    increment of wall time is the HW kernel latency, with host/RPC/launch
    overhead cancelled out.
    """
    nc = bass.Bass()
    # flat [B_PER*S, H] view of this core's hidden_state shard
    hidden = nc.declare_dram_parameter(
        "hidden", [B_PER * S, H], mybir.dt.float32, isOutput=False
    )
    # host-prepped [128, 68] bf16 mask layout (see module docstring)
    mask = nc.declare_dram_parameter(
        "mask", [PARTS, WCOLS], mybir.dt.bfloat16, isOutput=False
    )
    out = nc.declare_dram_parameter("out", [B_PER, H], mybir.dt.float32, isOutput=True)

    with (
        nc.sbuf_tensor([PARTS, WCOLS], mybir.dt.bfloat16) as work,
        nc.sbuf_tensor([PARTS, 1], mybir.dt.bfloat16) as ones,
        nc.sbuf_tensor([1, B_PER], mybir.dt.int32) as idx,
        nc.psum_tensor([1, B_PER, SEQC], mybir.dt.float32) as psum,
        nc.semaphore() as dma_sem,  # mask DMA completion -> PE
        nc.semaphore() as s_sem,  # store DMA completions -> SP/ACT tails
        nc.semaphore() as pe_sem,  # PE matmul -> DVE
        nc.semaphore() as v_sem,  # DVE reduce -> SP+ACT: idx ready
        nc.semaphore() as sel_sem,  # DVE ones memset -> PE (once)
        nc.Block() as block,
    ):

        @block.sync
        def _(sync):
            r0 = sync.alloc_register("r0")
            for i in range(reps):
                sync.dma_start(out=work[:], in_=mask[:]).then_inc(dma_sem, 16)
                # idx ready in SBUF
                sync.wait_ge(v_sem, i + 1)
                sync.reg_load(r0, idx[0:1, 0:1])
                # donate: the snap aliases the register instead of allocating
                # a fresh snapshot register per rep (the DMA descriptor
                # captures the value at issue, so reuse next rep is safe
                # behind the s_sem wait)
                v0 = sync.snap(r0, donate=True)
                sync.dma_start(
                    out=out[0:1, :], in_=hidden[bass.ds(v0, 1), :]
                ).then_inc(s_sem, 16)
                # don't let the program retire (or the next rep start)
                # with either store still in flight (scalar's included)
                sync.wait_ge(s_sem, 32 * (i + 1))

        @block.scalar
        def _(scalar):
            # second output row handled by ACT's sequencer in parallel with SP
            r1 = scalar.alloc_register("r1")
            for i in range(reps):
                scalar.wait_ge(v_sem, i + 1)
                # idx[0,1] already includes the +S shard-row offset, baked
                # into the bias columns on the host side
                scalar.reg_load(r1, idx[0:1, 1:2])
                v1 = scalar.snap(r1, donate=True)
                scalar.dma_start(
                    out=out[1:2, :], in_=hidden[bass.ds(v1, 1), :]
                ).then_inc(s_sem, 16)
                # keep the tail wait on ACT too: dropping it looks like a win
                # in the cost-model sim (-900ns) but measured ~300-400ns SLOWER
                # on hardware in interleaved A/B
                scalar.wait_ge(s_sem, 32 * (i + 1))

        @block.vector
        def _(vector):
            # ones column for the contraction; written once before the first
            # matmul (PE waits on sel_sem)
            vector.memset(ones[:], 1.0).then_inc(sel_sem, 1)
            for i in range(reps):
                vector.wait_ge(pe_sem, i + 1)
                # idx[0, b] = int32(sum(psum[0, b, :])) = len_b - 1; exact
                # (small integers in f32, converted on the DVE output stage).
                # One 3-D X-axis reduce handles both sequences and carries the
                # single sem update this walrus build allows per instruction.
                with nc.allow_low_precision(
                    reason="sum of 0/1 mask values; exact in int32"
                ):
                    vector.reduce_sum(
                        out=idx[0:1, 0:B_PER],
                        in_=psum[0:1, :, :],
                        axis=mybir.AxisListType.X,
                    ).then_inc(v_sem, 1)

        @block.tensor
        def _(tensor):
            tensor.wait_ge(sel_sem, 1)
            for i in range(reps):
                tensor.wait_ge(dma_sem, 16 * (i + 1))
                # psum[0, b, c] = sum_p work[p, 34b+c]: 68 column sums
                nc.tensor.matmul(
                    out=psum[0:1, :, :],
                    lhsT=ones[:],
                    rhs=work[:],
                    start=True,
                    stop=True,
                ).then_inc(pe_sem, 1)

    return nc


def build_bass_loop(n_iters: int) -> bass.Bass:
    """Timing build: the unrolled cumulative-threshold chain.  Semaphore
    counters were verified on-device not to wrap at 16 bits (reps=4000,
    s_sem up to 128000, bit-exact result), so plain build_bass(reps=N)
    serves for arbitrary N."""
    return build_bass(reps=n_iters)


def _get_nc() -> bass.Bass:
    global _NC_CACHE
    if _NC_CACHE is None:
        _NC_CACHE = build_bass()
    return _NC_CACHE


def _prep_mask(mask_rows: np.ndarray) -> np.ndarray:
    """[B_PER, S] 0/1 mask -> [128, 68] bf16 tile (see module docstring)."""
    bf16 = mybir.dt.np(mybir.dt.bfloat16)
    tile = np.zeros((PARTS, WCOLS), dtype=bf16)
    for b in range(B_PER):
        m = np.asarray(mask_rows[b]).reshape(PARTS, CHUNK).astype(bf16)
        tile[:, b * SEQC : b * SEQC + CHUNK] = m
        # bias columns: b*S - 1 split into bf16-exact terms
        tile[0, b * SEQC + CHUNK] = float(b * S)  # 0.0 or 4096.0
        tile[0, b * SEQC + CHUNK + 1] = -1.0
    return np.ascontiguousarray(tile)


def _shard_inputs(hidden_state: np.ndarray, attention_mask: np.ndarray):
    in_maps = []
    for c in range(N_CORES):
        lo, hi = c * B_PER, (c + 1) * B_PER
        hs = np.ascontiguousarray(
            hidden_state[lo:hi].reshape(B_PER * S, H), dtype=np.float32
        )
        in_maps.append({"hidden": hs, "mask": _prep_mask(attention_mask[lo:hi])})
    return in_maps


def run(hidden_state: np.ndarray, attention_mask: np.ndarray, **spmd_kwargs):
    """Run on 8 NeuronCores; returns (full_output, BassKernelResults)."""
    nc = _get_nc()
    in_maps = _shard_inputs(np.asarray(hidden_state), np.asarray(attention_mask))
    res = run_bass_kernel_spmd(nc, in_maps, core_ids=list(range(N_CORES)), **spmd_kwargs)
    out = np.concatenate([r["out"] for r in res.results], axis=0)
    return out, res


def kernel(hidden_state: np.ndarray, attention_mask: np.ndarray) -> np.ndarray:
    out, _ = run(hidden_state, attention_mask)
    return out
